# revision 4
# baseline (speedup 1.0000x reference)
"""Trainium2 Bass kernel for nn_FFTChainMatrix (block-circulant matmul via 64-pt rFFT).

y = x @ W.T with W 4096x4096 block-circulant, computed in the FFT domain as a
3-matmul pipeline (rfft -> per-freq contraction -> irfft) with two SBUF
per-frequency shuffle DMA stages between the matmuls.

v2 (vs v1 baseline at ~164us):
  - x/y live in DRAM in the exact SBUF layout (host pre/post-transposes), so
    loads/stores are fully-contiguous DMAs (8-32 KiB per partition row)
    instead of 1 KiB-run gathers that ran at ~17 GB/s.
  - DMA issue spread over 3 queues (sync/gpsimd/scalar HWDGE).
  - loads chased by S1, stores chase S3; warm matmuls keep the PE p-state
    hot across the shuffle barriers.

Per-core data layout (T=512 tokens/core, f16):
  x_sb [128=(64j+d), ib*512 + t]                (in-block i = 2*ib + j)
  S1: out = A2.T @ x   -> X1 [128=(4fp+2z+j), (ib,t)]
  shuf fp: X2[(2z+j)*32+ib, fp*512+t] = X1[4fp+2z+j, ib*512+t]
  S2: out = G[fp].T @ X2 -> Y2 [128=(4ob+2zo+jo), (fp,t)]
  unshuf ob: Y3[(2zo+jo)*32+fp, ob*512+t] = Y2[4ob+2zo+jo, fp*512+t]
  S3: out = B2.T @ Y3 -> ys [128=(64jo+d), (ob,t)]   (out o = 2*ob + jo)

Sharding: data-parallel over tokens, 4096 tokens -> 8 cores x 512.
"""

from contextlib import ExitStack

import numpy as np

BLK = 64
NB = 64           # circulant blocks per side
T = 512           # tokens per core
NCORES = 8
FEAT = 4096
NC_COLS = 32 * T  # 16384 sbuf cols per full tile


# ---------------------------------------------------------------- host math
def _build_matrices(circulant_params, channel_weights):
    """A2 [128,128], G [32,128,128], B2 [128,128] (float64 math)."""
    c_w = np.einsum(
        "m,moid->oid",
        np.asarray(channel_weights, np.float64),
        np.asarray(circulant_params, np.float64),
    )
    Chat = np.fft.rfft(c_w, axis=-1)
    Wr, Wi = Chat.real, Chat.imag

    r = np.arange(BLK)
    A64 = np.zeros((BLK, BLK))
    A64[0, :] = 1.0
    A64[1, :] = (-1.0) ** r
    B64 = np.zeros((BLK, BLK))
    B64[:, 0] = 1.0 / BLK
    B64[:, 1] = ((-1.0) ** r) / BLK
    for p in range(1, 32):
        cc = np.cos(2 * np.pi * p * r / BLK)
        ss = np.sin(2 * np.pi * p * r / BLK)
        A64[2 * p, :] = cc
        A64[2 * p + 1, :] = -ss
        B64[:, 2 * p] = 2.0 * cc / BLK
        B64[:, 2 * p + 1] = -2.0 * ss / BLK

    # A2[(64j + d), (4fp + 2z + j)] = A64[2fp+z, d]
    # B2[(2zo + jo)*32 + fp, (64jo + d)] = B64[d, 2fp + zo]
    A2 = np.zeros((128, 128))
    B2 = np.zeros((128, 128))
    for j in range(2):
        A2[64 * j: 64 * j + 64, j::2] = A64.T
    for zo in range(2):
        for jo in range(2):
            for fp in range(32):
                B2[(2 * zo + jo) * 32 + fp, 64 * jo: 64 * jo + 64] = \
                    B64[:, 2 * fp + zo]

    # G[fp][((2z + ji)*32 + ib), (4 ob + 2 zo + jo)]   (i = 2 ib + ji)
    # z/zo: 0 = Re(F_fp), 1 = Im(F_fp)  (for fp=0: 0 = F_0, 1 = F_32, both real)
    i = np.arange(NB)
    rows = (2 * np.arange(2)[None, :] + (i % 2)[:, None]) * 32 + (i // 2)[:, None]
    G = np.zeros((32, 128, 128))
    blk = np.zeros((NB, 2, NB, 2))
    for fp in range(32):
        blk[:] = 0.0
        if fp == 0:
            blk[:, 0, :, 0] = Wr[:, :, 0].T
            blk[:, 1, :, 1] = Wr[:, :, 32].T
        else:
            blk[:, 0, :, 0] = Wr[:, :, fp].T
            blk[:, 1, :, 0] = -Wi[:, :, fp].T
            blk[:, 0, :, 1] = Wi[:, :, fp].T
            blk[:, 1, :, 1] = Wr[:, :, fp].T
        cols = 4 * (i // 2)[:, None] + 2 * np.arange(2)[None, :] + (i % 2)[:, None]
        G[fp][rows[:, :, None, None], cols[None, None, :, :]] = blk
    return A2, G, B2


# ---------------------------------------------------------------- bass trace
def _trace_nc():
    import concourse.bass as bass  # noqa: F401
    import concourse.mybir as mybir
    import concourse.tile as tile
    from concourse import bacc

    f32 = mybir.dt.float32
    f16 = mybir.dt.float16

    nc = bacc.Bacc("TRN2", target_bir_lowering=False, debug=False,
                   num_devices=NCORES)
    x_h = nc.dram_tensor("x_shard", [128, NC_COLS], f16,
                         kind="ExternalInput").ap()
    w_h = nc.dram_tensor("w_mats", [128, 4352], f16, kind="ExternalInput").ap()
    y_h = nc.dram_tensor("y_shard", [128, NC_COLS], f16,
                         kind="ExternalOutput").ap()

    # [vector, scalar] modeled busy ns (scalar also pays for its DMA issues)
    eng_cost = [0.0, 0.0]
    dma_ix = [0]

    with tile.TileContext(nc) as tc, ExitStack() as ctx:
        wpool = ctx.enter_context(tc.tile_pool(name="weights", bufs=1))
        xpool = ctx.enter_context(tc.tile_pool(name="xin", bufs=4))
        x1pool = ctx.enter_context(tc.tile_pool(name="x1u", bufs=1))
        x2pool = ctx.enter_context(tc.tile_pool(name="x2sb", bufs=1))
        y2pool = ctx.enter_context(tc.tile_pool(name="y2u", bufs=1))
        y3pool = ctx.enter_context(tc.tile_pool(name="y3sb", bufs=1))
        ypool = ctx.enter_context(tc.tile_pool(name="yout", bufs=4))
        wmpool = ctx.enter_context(tc.tile_pool(name="warm", bufs=1))
        mmps = ctx.enter_context(tc.tile_pool(name="mmps", bufs=7, space="PSUM"))

        # PSUM->SBUF copyback: only DVE/Act can read PSUM.  Greedy-balance
        # by modeled per-op cost (scalar's DMA issue time is also tracked).
        def cb(dst, src, n=512):
            cost_v = n * 1.04 + 125.0
            cost_s = n / 1.2 + 143.0
            if eng_cost[0] + cost_v <= eng_cost[1] + cost_s:
                eng_cost[0] += cost_v
                nc.vector.tensor_copy(dst, src)
            else:
                eng_cost[1] += cost_s
                nc.scalar.copy(dst, src)

        def dma(dst, src):
            eng = (nc.sync, nc.gpsimd)[dma_ix[0] % 2]
            dma_ix[0] += 1
            eng.dma_start(dst, src)

        def dma3(dst, src):
            # shuffle stages: spread issue over 3 queues; cb is idle there,
            # so scalar can afford every 3rd issue.
            k = dma_ix[0] % 3
            dma_ix[0] += 1
            if k == 2:
                eng_cost[1] += 700.0
                nc.scalar.dma_start(dst, src)
            else:
                (nc.sync, nc.gpsimd)[k].dma_start(dst, src)

        # ---- weight + x loads (x in 4 contiguous 1 MiB chunks of 8 ibs)
        wt = wpool.tile([128, 4352], f16)
        nc.sync.dma_start(wt[:], w_h[:])
        a2 = wt[:, 4096:4224]
        b2 = wt[:, 4224:4352]

        xsb = [xpool.tile([128, 8 * T], f16, tag="xsb", name=f"xsb{h}")
               for h in range(4)]
        for h in range(4):
            dma(xsb[h][:], x_h[:, h * 8 * T:(h + 1) * 8 * T])

        x1u = x1pool.tile([128, NC_COLS], f16)
        x2sb = x2pool.tile([128, NC_COLS], f16)
        y2u = y2pool.tile([128, NC_COLS], f16)
        y3sb = y3pool.tile([128, NC_COLS], f16)
        ys = [ypool.tile([128, 8 * T], f16, tag="ys", name=f"ys{h}")
              for h in range(4)]

        # ---- PE warm stream: ramp the HAM p-state while the x loads land,
        # and keep it hot across the shuffle barriers.
        warm = wmpool.tile([128, 512], f16)
        nc.vector.memset(warm[:], 0.0)

        def warm_mm(n):
            for _ in range(n):
                ps = mmps.tile([128, 512], f32, tag="mm")
                nc.tensor.matmul(ps[:], warm[:, 0:128], warm[:],
                                 start=True, stop=True)

        warm_mm(10)

        # ---- S1 (rfft): stationary A2, moving x[ib]; chases the loads
        for ib in range(32):
            ps = mmps.tile([128, 512], f32, tag="mm")
            nc.tensor.matmul(ps[:], a2, xsb[ib // 8][:, (ib % 8) * T:
                                                     (ib % 8 + 1) * T],
                             start=True, stop=True)
            cb(x1u[:, ib * T:(ib + 1) * T], ps[:])
        warm_mm(12)

        # ---- shuffle: X1 rows 4fp..4fp+4 across all ib -> X2[:, fp*T+t]
        for fp in range(32):
            src = x1u[4 * fp:4 * fp + 4, :].rearrange("p (ib t) -> p ib t", t=T)
            dma3(x2sb[:, fp * T:(fp + 1) * T], src)

        # ---- S2: per-freq-pair complex contraction; chases the shuffle
        for fp in range(32):
            ps = mmps.tile([128, 512], f32, tag="mm")
            nc.tensor.matmul(ps[:], wt[:, fp * 128:(fp + 1) * 128],
                             x2sb[:, fp * T:(fp + 1) * T],
                             start=True, stop=True)
            cb(y2u[:, fp * T:(fp + 1) * T], ps[:])
        warm_mm(12)

        # ---- unshuffle: Y2 rows 4ob..4ob+4 -> Y3[:, ob*T+t]
        for ob in range(32):
            src = y2u[4 * ob:4 * ob + 4, :].rearrange("p (f t) -> p f t", t=T)
            dma3(y3sb[:, ob * T:(ob + 1) * T], src)

        # ---- S3 (irfft) + stores chase in 4 chunks of 8 obs
        for ob in range(32):
            ps = mmps.tile([128, 512], f32, tag="mm")
            nc.tensor.matmul(ps[:], b2, y3sb[:, ob * T:(ob + 1) * T],
                             start=True, stop=True)
            cb(ys[ob // 8][:, (ob % 8) * T:(ob % 8 + 1) * T], ps[:])
            if ob % 8 == 7:
                h = ob // 8
                dma(y_h[:, h * 8 * T:(h + 1) * 8 * T], ys[h][:])

    nc.compile()
    return nc


_CACHE = {}


def make_in_maps(x, circulant_params, channel_weights):
    xf = np.asarray(x, np.float32).reshape(-1, FEAT)
    assert xf.shape[0] == NCORES * T, f"unexpected token count {xf.shape}"
    A2, G, B2 = _build_matrices(circulant_params, channel_weights)
    w = np.zeros((128, 4352), np.float16)
    w[:, 0:4096] = G.transpose(1, 0, 2).reshape(128, 4096).astype(np.float16)
    w[:, 4096:4224] = A2.astype(np.float16)
    w[:, 4224:4352] = B2.astype(np.float16)
    # x_dev[c][64j+d, ib*512 + t] = x[c*512 + t, 64*(2ib+j) + d]
    xd = xf.astype(np.float16).reshape(NCORES, T, 32, 2, 64)
    xd = np.ascontiguousarray(xd.transpose(0, 3, 4, 2, 1)).reshape(
        NCORES, 128, NC_COLS)
    return [
        {"x_shard": xd[c], "w_mats": w}
        for c in range(NCORES)
    ]


def kernel(x, circulant_params, channel_weights):
    from concourse.bass_utils import run_bass_kernel_spmd

    x = np.asarray(x, np.float32)
    orig_shape = x.shape

    if "nc" not in _CACHE:
        _CACHE["nc"] = _trace_nc()
    nc = _CACHE["nc"]

    in_maps = make_in_maps(x, circulant_params, channel_weights)
    res = run_bass_kernel_spmd(nc, in_maps, core_ids=list(range(NCORES)))
    # y_dev[c][64jo+d, ob*512 + t] = y[c*512 + t, 128*ob + 64*jo + d]
    yd = np.stack([res.results[c]["y_shard"] for c in range(NCORES)])
    yd = yd.reshape(NCORES, 2, 64, 32, T).transpose(0, 4, 3, 1, 2)
    return np.ascontiguousarray(yd).reshape(orig_shape).astype(np.float32)


# revision 5
# speedup vs baseline: 1.5724x; 1.5724x over previous
"""Trainium2 Bass kernel for nn_FFTChainMatrix (block-circulant matmul via 64-pt rFFT).

y = x @ W.T with W 4096x4096 block-circulant, computed in the FFT domain as a
3-matmul pipeline (rfft -> per-freq contraction -> irfft) with two SBUF
per-frequency shuffle DMA stages between the matmuls.

v2 (vs v1 baseline at ~164us):
  - x/y live in DRAM in the exact SBUF layout (host pre/post-transposes), so
    loads/stores are fully-contiguous DMAs (8-32 KiB per partition row)
    instead of 1 KiB-run gathers that ran at ~17 GB/s.
  - DMA issue spread over 3 queues (sync/gpsimd/scalar HWDGE).
  - loads chased by S1, stores chase S3; warm matmuls keep the PE p-state
    hot across the shuffle barriers.

Per-core data layout (T=512 tokens/core, f16):
  x_sb [128=(64j+d), ib*512 + t]                (in-block i = 2*ib + j)
  S1: out = A2.T @ x   -> X1 [128=cport(fp, 2z+j), (ib,t)]
  shuf fp: X2[(2z+j)*32+ib, fp*512+t] = X1[cport(fp,2z+j), ib*512+t]
           (cport(b,q) = 16*(b//4) + 4q + b%4: stride-4 partitions -> 4 AXI ports)
  S2: out = G[fp].T @ X2 -> Y2 [128=cport(ob, 2zo+jo), (fp,t)]
  unshuf ob: Y3[(2zo+jo)*32+fp, ob*512+t] = Y2[cport(ob,2zo+jo), fp*512+t]
  S3: out = B2.T @ Y3 -> ys [128=(64jo+d), (ob,t)]   (out o = 2*ob + jo)

Sharding: data-parallel over tokens, 4096 tokens -> 8 cores x 512.
"""

from contextlib import ExitStack

import numpy as np

BLK = 64
NB = 64           # circulant blocks per side
T = 512           # tokens per core
NCORES = 8
FEAT = 4096
NC_COLS = 32 * T  # 16384 sbuf cols per full tile


# ---------------------------------------------------------------- host math
def _build_matrices(circulant_params, channel_weights):
    """A2 [128,128], G [32,128,128], B2 [128,128] (float64 math)."""
    c_w = np.einsum(
        "m,moid->oid",
        np.asarray(channel_weights, np.float64),
        np.asarray(circulant_params, np.float64),
    )
    Chat = np.fft.rfft(c_w, axis=-1)
    Wr, Wi = Chat.real, Chat.imag

    r = np.arange(BLK)
    A64 = np.zeros((BLK, BLK))
    A64[0, :] = 1.0
    A64[1, :] = (-1.0) ** r
    B64 = np.zeros((BLK, BLK))
    B64[:, 0] = 1.0 / BLK
    B64[:, 1] = ((-1.0) ** r) / BLK
    for p in range(1, 32):
        cc = np.cos(2 * np.pi * p * r / BLK)
        ss = np.sin(2 * np.pi * p * r / BLK)
        A64[2 * p, :] = cc
        A64[2 * p + 1, :] = -ss
        B64[:, 2 * p] = 2.0 * cc / BLK
        B64[:, 2 * p + 1] = -2.0 * ss / BLK

    # Spread-port column index: the 4 source partitions of one shuffle DMA
    # land on 4 distinct SBUF AXI ports (stride-4 partitions) instead of 1.
    def cport(blk4, q):
        return 16 * (blk4 // 4) + 4 * q + (blk4 % 4)

    # A2[(64j + d), cport(fp, 2z+j)] = A64[2fp+z, d]
    # B2[(2zo + jo)*32 + fp, (64jo + d)] = B64[d, 2fp + zo]
    A2 = np.zeros((128, 128))
    B2 = np.zeros((128, 128))
    for j in range(2):
        for z in range(2):
            for fp in range(32):
                A2[64 * j: 64 * j + 64, cport(fp, 2 * z + j)] = A64[2 * fp + z, :]
    for zo in range(2):
        for jo in range(2):
            for fp in range(32):
                B2[(2 * zo + jo) * 32 + fp, 64 * jo: 64 * jo + 64] = \
                    B64[:, 2 * fp + zo]

    # G[fp][((2z + ji)*32 + ib), (4 ob + 2 zo + jo)]   (i = 2 ib + ji)
    # z/zo: 0 = Re(F_fp), 1 = Im(F_fp)  (for fp=0: 0 = F_0, 1 = F_32, both real)
    i = np.arange(NB)
    rows = (2 * np.arange(2)[None, :] + (i % 2)[:, None]) * 32 + (i // 2)[:, None]
    G = np.zeros((32, 128, 128))
    blk = np.zeros((NB, 2, NB, 2))
    for fp in range(32):
        blk[:] = 0.0
        if fp == 0:
            blk[:, 0, :, 0] = Wr[:, :, 0].T
            blk[:, 1, :, 1] = Wr[:, :, 32].T
        else:
            blk[:, 0, :, 0] = Wr[:, :, fp].T
            blk[:, 1, :, 0] = -Wi[:, :, fp].T
            blk[:, 0, :, 1] = Wi[:, :, fp].T
            blk[:, 1, :, 1] = Wr[:, :, fp].T
        cols = (16 * ((i // 2) // 4) + ((i // 2) % 4))[:, None] + \
            4 * (2 * np.arange(2)[None, :] + (i % 2)[:, None])
        G[fp][rows[:, :, None, None], cols[None, None, :, :]] = blk
    return A2, G, B2


# ---------------------------------------------------------------- bass trace
def _trace_nc():
    import concourse.bass as bass  # noqa: F401
    import concourse.mybir as mybir
    import concourse.tile as tile
    from concourse import bacc

    f32 = mybir.dt.float32
    f16 = mybir.dt.float16

    nc = bacc.Bacc("TRN2", target_bir_lowering=False, debug=False,
                   num_devices=NCORES)
    x_h = nc.dram_tensor("x_shard", [128, NC_COLS], f16,
                         kind="ExternalInput").ap()
    w_h = nc.dram_tensor("w_mats", [128, 4352], f16, kind="ExternalInput").ap()
    y_h = nc.dram_tensor("y_shard", [128, NC_COLS], f16,
                         kind="ExternalOutput").ap()

    # [vector, scalar] modeled busy ns (scalar also pays for its DMA issues)
    eng_cost = [0.0, 0.0]
    dma_ix = [0]

    with tile.TileContext(nc) as tc, ExitStack() as ctx:
        wpool = ctx.enter_context(tc.tile_pool(name="weights", bufs=1))
        xpool = ctx.enter_context(tc.tile_pool(name="xin", bufs=4))
        x1pool = ctx.enter_context(tc.tile_pool(name="x1u", bufs=1))
        x2pool = ctx.enter_context(tc.tile_pool(name="x2sb", bufs=1))
        y2pool = ctx.enter_context(tc.tile_pool(name="y2u", bufs=1))
        y3pool = ctx.enter_context(tc.tile_pool(name="y3sb", bufs=1))
        ypool = ctx.enter_context(tc.tile_pool(name="yout", bufs=4))
        wmpool = ctx.enter_context(tc.tile_pool(name="warm", bufs=1))
        mmps = ctx.enter_context(tc.tile_pool(name="mmps", bufs=7, space="PSUM"))

        # PSUM->SBUF copyback: only DVE/Act can read PSUM.  Greedy-balance
        # by modeled per-op cost (scalar's DMA issue time is also tracked).
        def cb(dst, src, n=512):
            cost_v = n * 1.04 + 125.0
            cost_s = n / 1.2 + 143.0
            if eng_cost[0] + cost_v <= eng_cost[1] + cost_s:
                eng_cost[0] += cost_v
                nc.vector.tensor_copy(dst, src)
            else:
                eng_cost[1] += cost_s
                nc.scalar.copy(dst, src)

        def dma(dst, src):
            eng = (nc.sync, nc.gpsimd)[dma_ix[0] % 2]
            dma_ix[0] += 1
            eng.dma_start(dst, src)

        # ---- weight + x loads (x in 4 contiguous 1 MiB chunks of 8 ibs)
        wt = wpool.tile([128, 4352], f16)
        nc.sync.dma_start(wt[:], w_h[:])
        a2 = wt[:, 4096:4224]
        b2 = wt[:, 4224:4352]

        xsb = [xpool.tile([128, 8 * T], f16, tag="xsb", name=f"xsb{h}")
               for h in range(4)]
        for h in range(4):
            dma(xsb[h][:], x_h[:, h * 8 * T:(h + 1) * 8 * T])

        x1u = x1pool.tile([128, NC_COLS], f16)
        x2sb = x2pool.tile([128, NC_COLS], f16)
        y2u = y2pool.tile([128, NC_COLS], f16)
        y3sb = y3pool.tile([128, NC_COLS], f16)
        ys = [ypool.tile([128, 8 * T], f16, tag="ys", name=f"ys{h}")
              for h in range(4)]

        # ---- PE warm stream: ramp the HAM p-state while the x loads land,
        # and keep it hot across the shuffle barriers.
        warm = wmpool.tile([128, 512], f16)
        nc.vector.memset(warm[:], 0.0)

        def warm_mm(n):
            for _ in range(n):
                ps = mmps.tile([128, 512], f32, tag="mm")
                nc.tensor.matmul(ps[:], warm[:, 0:128], warm[:],
                                 start=True, stop=True)

        warm_mm(10)

        # ---- S1 (rfft): stationary A2, moving x[ib]; chases the loads
        for ib in range(32):
            ps = mmps.tile([128, 512], f32, tag="mm")
            nc.tensor.matmul(ps[:], a2, xsb[ib // 8][:, (ib % 8) * T:
                                                     (ib % 8 + 1) * T],
                             start=True, stop=True)
            cb(x1u[:, ib * T:(ib + 1) * T], ps[:])
        warm_mm(12)

        # port-interleaved issue order: consecutive DMAs read disjoint port sets
        PORT_ORDER = [b + 16 * c + 4 * a
                      for b in range(4) for c in range(2) for a in (0, 1)]
        PORT_ORDER += [8 + v for v in PORT_ORDER]

        # ---- shuffle: X1 partitions cport(fp, q) across all ib -> X2[:, fp*T+t]
        for fp in PORT_ORDER:
            p0 = 16 * (fp // 4) + (fp % 4)
            src = x1u[p0:p0 + 13:4, :].rearrange("p (ib t) -> p ib t", t=T)
            dma(x2sb[:, fp * T:(fp + 1) * T], src)

        # ---- S2: per-freq-pair complex contraction; chases the shuffle
        for fp in PORT_ORDER:
            ps = mmps.tile([128, 512], f32, tag="mm")
            nc.tensor.matmul(ps[:], wt[:, fp * 128:(fp + 1) * 128],
                             x2sb[:, fp * T:(fp + 1) * T],
                             start=True, stop=True)
            cb(y2u[:, fp * T:(fp + 1) * T], ps[:])
        warm_mm(12)

        # ---- unshuffle: Y2 partitions cport(ob, w) -> Y3[:, ob*T+t]
        for ob in PORT_ORDER:
            p0 = 16 * (ob // 4) + (ob % 4)
            src = y2u[p0:p0 + 13:4, :].rearrange("p (f t) -> p f t", t=T)
            dma(y3sb[:, ob * T:(ob + 1) * T], src)

        # ---- S3 (irfft) + stores chase in 4 chunks of 8 obs
        done = set()
        for ob in PORT_ORDER:
            ps = mmps.tile([128, 512], f32, tag="mm")
            nc.tensor.matmul(ps[:], b2, y3sb[:, ob * T:(ob + 1) * T],
                             start=True, stop=True)
            cb(ys[ob // 8][:, (ob % 8) * T:(ob % 8 + 1) * T], ps[:])
            done.add(ob)
            h = ob // 8
            if all(8 * h + k in done for k in range(8)):
                nc.sync.dma_start(y_h[:, h * 8 * T:(h + 1) * 8 * T], ys[h][:])

    nc.compile()
    return nc


_CACHE = {}


def make_in_maps(x, circulant_params, channel_weights):
    xf = np.asarray(x, np.float32).reshape(-1, FEAT)
    assert xf.shape[0] == NCORES * T, f"unexpected token count {xf.shape}"
    A2, G, B2 = _build_matrices(circulant_params, channel_weights)
    w = np.zeros((128, 4352), np.float16)
    w[:, 0:4096] = G.transpose(1, 0, 2).reshape(128, 4096).astype(np.float16)
    w[:, 4096:4224] = A2.astype(np.float16)
    w[:, 4224:4352] = B2.astype(np.float16)
    # x_dev[c][64j+d, ib*512 + t] = x[c*512 + t, 64*(2ib+j) + d]
    xd = xf.astype(np.float16).reshape(NCORES, T, 32, 2, 64)
    xd = np.ascontiguousarray(xd.transpose(0, 3, 4, 2, 1)).reshape(
        NCORES, 128, NC_COLS)
    return [
        {"x_shard": xd[c], "w_mats": w}
        for c in range(NCORES)
    ]


def kernel(x, circulant_params, channel_weights):
    from concourse.bass_utils import run_bass_kernel_spmd

    x = np.asarray(x, np.float32)
    orig_shape = x.shape

    if "nc" not in _CACHE:
        _CACHE["nc"] = _trace_nc()
    nc = _CACHE["nc"]

    in_maps = make_in_maps(x, circulant_params, channel_weights)
    res = run_bass_kernel_spmd(nc, in_maps, core_ids=list(range(NCORES)))
    # y_dev[c][64jo+d, ob*512 + t] = y[c*512 + t, 128*ob + 64*jo + d]
    yd = np.stack([res.results[c]["y_shard"] for c in range(NCORES)])
    yd = yd.reshape(NCORES, 2, 64, 32, T).transpose(0, 4, 3, 1, 2)
    return np.ascontiguousarray(yd).reshape(orig_shape).astype(np.float32)
